# revision 20
# baseline (speedup 1.0000x reference)
"""Multi-head attention (B=2, S=2048, H=1024, 16 heads) on 8 trn2 NeuronCores.

Sharding: 2-way batch x 4-way head tensor parallel. Core c owns batch c//4 and
heads 4*(c%4) .. 4*(c%4)+4 (256 channels of the QKV projections, 256 input
channels of the output projection). Each core consumes its batch's activations
(transposed, bf16) and returns a bf16 partial of the wo projection; the host
sums the 4 partials per batch and adds the bias.

The attention inner loop is bound by the scalar-engine exp (~1.05us per
128-key step), not the PE, so the kernel keeps only the minimum projection
work ahead of the attention stream (K/Q for head-pair 0, V for heads 0-1,
DMA-paced f-outer) and interleaves the remaining projections (Q/K pair 1,
V heads 2-3) into the attention steps to fill the PE idle and keep the HAM
clock warm. attnV lags scores/exp by LAG steps; wo is deferred LAG_WO steps
so the softmax-norm chain (reciprocal+broadcast) runs in the shadow of the
next unit.
"""

import os
import threading

import numpy as np
import ml_dtypes

import concourse.bass as bass
import concourse.mybir as mybir
import concourse.tile as tile
from concourse import bacc
from concourse.bass_utils import run_bass_kernel_spmd

BF16 = ml_dtypes.bfloat16
F32 = mybir.dt.float32
BF = mybir.dt.bfloat16

B = 2
S = 2048            # tokens per core (one batch)
H = 1024
NH_LOCAL = 4        # heads per core
HD = 64
CPC = NH_LOCAL * HD  # 256 channels per core
NF = H // 128       # feature chunks of the input dim
N_CORES = 8
LAG = 3             # attnV lag (in key-block steps) behind scores/exp
LAG_WO = 8          # wo deferral (steps) to hide the softmax-norm latency

_cache = threading.Lock()
_nc = None

LAST_RESULT = None  # BassKernelResults of the most recent run (for test.py)


def _build_nc():
    nc = bacc.Bacc(None, target_bir_lowering=False, debug=False)

    xq_d = nc.dram_tensor("xq_t", [H, S], BF, kind="ExternalInput")
    xk_d = nc.dram_tensor("xk_t", [H, S], BF, kind="ExternalInput")
    xv_d = nc.dram_tensor("xv_t", [H, S], BF, kind="ExternalInput")
    wq_d = nc.dram_tensor("wq_t", [H, CPC], BF, kind="ExternalInput")
    wk_d = nc.dram_tensor("wk_t", [H, CPC], BF, kind="ExternalInput")
    wv_d = nc.dram_tensor("wv_t", [H, CPC], BF, kind="ExternalInput")
    bq_d = nc.dram_tensor("bq", [CPC, 1], F32, kind="ExternalInput")
    bk_d = nc.dram_tensor("bk", [CPC, 1], F32, kind="ExternalInput")
    bv_d = nc.dram_tensor("bv", [1, CPC], BF, kind="ExternalInput")
    wo_d = nc.dram_tensor("wo_t", [CPC, H], BF, kind="ExternalInput")
    y_d = nc.dram_tensor("y_t", [H, S], BF, kind="ExternalOutput")

    xq_ap = xq_d.rearrange("(nf p) s -> nf p s", p=128)
    xk_ap = xk_d.rearrange("(nf p) s -> nf p s", p=128)
    xv_ap = xv_d.rearrange("(nf p) s -> nf p s", p=128)
    wq_ap = wq_d.rearrange("(nf p) c -> nf p c", p=128)
    wk_ap = wk_d.rearrange("(nf p) c -> nf p c", p=128)
    wv_ap = wv_d.rearrange("(nf p) c -> nf p c", p=128)
    y_ap = y_d.rearrange("(no p) s -> no p s", p=128)

    Exp = mybir.ActivationFunctionType.Exp
    Copy = mybir.ActivationFunctionType.Identity

    NSI = S // 128   # 16 key blocks
    NQS = S // 512   # 4 query slices

    with tile.TileContext(nc) as tc:
        with (
            tc.tile_pool(name="const", bufs=1) as const,
            tc.tile_pool(name="res", bufs=1) as res,
            tc.tile_pool(name="psum", bufs=1, space="PSUM") as psum,
            tc.tile_pool(name="epool", bufs=6) as epool,
            tc.tile_pool(name="npool", bufs=2) as npool,
            tc.tile_pool(name="onpool", bufs=2) as onpool,
            tc.tile_pool(name="ypool", bufs=3) as ypool,
            tc.tile_pool(name="xin", bufs=8) as xin,
        ):
            # --- input chunk DMAs first (K, Q, V order), on the SP queue ---
            xtk, xtq, xtv = [], [], []
            for f in range(NF):
                t = xin.tile([128, S], BF, tag="xk", name=f"xk{f}")
                nc.sync.dma_start(t[:], xk_ap[f])
                xtk.append(t)
            for f in range(NF):
                t = xin.tile([128, S], BF, tag="xq", name=f"xq{f}")
                nc.sync.dma_start(t[:], xq_ap[f])
                xtq.append(t)
            for f in range(NF):
                t = xin.tile([128, S], BF, tag="xv", name=f"xv{f}")
                nc.sync.dma_start(t[:], xv_ap[f])
                xtv.append(t)

            # --- weights (per-chunk, on the ACT/gpsimd issue queues) ---
            wq_sb = const.tile([128, NF, CPC], BF)
            wk_sb = const.tile([128, NF, CPC], BF)
            wv_sb = const.tile([128, NF, CPC], BF)
            wo_sb = const.tile([128, 2, NF, 128], BF)
            bq_sb = const.tile([128, 2], F32)
            bk_sb = const.tile([128, 2], F32)
            bv_sb = const.tile([1, CPC], BF)
            ones1 = const.tile([1, 128], BF)
            for f in range(NF):
                nc.scalar.dma_start(wk_sb[:, f, :], wk_ap[f])
            nc.scalar.dma_start(
                bk_sb[:], bk_d.rearrange("(hp p) one -> p (hp one)", p=128)
            )
            for f in range(NF):
                nc.scalar.dma_start(wq_sb[:, f, :], wq_ap[f])
            nc.scalar.dma_start(
                bq_sb[:], bq_d.rearrange("(hp p) one -> p (hp one)", p=128)
            )
            for f in range(NF):
                nc.gpsimd.dma_start(wv_sb[:, f, :], wv_ap[f])
            nc.gpsimd.dma_start(bv_sb[:], bv_d[:])
            nc.scalar.dma_start(
                wo_sb[:], wo_d.rearrange("(hp p) (no c) -> p hp no c", p=128, c=128)
            )
            nc.gpsimd.memset(ones1[:], 1.0)

            # --- residents ---
            QT = [res.tile([128, S], BF, name=f"QT{hp}") for hp in range(2)]
            KT = [res.tile([128, S], BF, name=f"KT{hp}") for hp in range(2)]
            V = res.tile([128, NH_LOCAL, NSI, HD + 1], BF)
            nc.gpsimd.memset(V[:, :, :, HD : HD + 1], 1.0)

            # --- projection emitters ---
            def emit_qk_outer(xt, w_sb, b_sb, out_t, hp, pname):
                # f-outer over 4 live psum groups: starts as soon as the
                # first input chunk lands (DMA-paced)
                cs = slice(hp * 128, (hp + 1) * 128)
                pst = []
                for sw in range(NQS):
                    tag = ("s", "s", "po0", "po1")[sw]
                    pst.append(psum.tile([128, 512], F32, tag=tag, bufs=2,
                                         name=f"pj{pname}{sw}"))
                for f in range(NF):
                    for sw in range(NQS):
                        nc.tensor.matmul(
                            pst[sw][:],
                            lhsT=w_sb[:, f, cs],
                            rhs=xt[f][:, sw * 512 : (sw + 1) * 512],
                            start=(f == 0),
                            stop=(f == NF - 1),
                        )
                for sw in range(NQS):
                    nc.scalar.activation(
                        out_t[hp][:, sw * 512 : (sw + 1) * 512], pst[sw][:],
                        Copy, bias=b_sb[:, hp : hp + 1],
                    )

            def emit_qk_group(xt, w_sb, b_sb, out_t, hp, sw, pname):
                # single f-inner group (used for the interleaved pair-1 work)
                cs = slice(hp * 128, (hp + 1) * 128)
                ps = psum.tile([128, 512], F32, tag="s", bufs=2,
                               name=f"pg{pname}{hp}{sw}")
                for f in range(NF):
                    nc.tensor.matmul(
                        ps[:],
                        lhsT=w_sb[:, f, cs],
                        rhs=xt[f][:, sw * 512 : (sw + 1) * 512],
                        start=(f == 0),
                        stop=(f == NF - 1),
                    )
                nc.scalar.activation(
                    out_t[hp][:, sw * 512 : (sw + 1) * 512], ps[:], Copy,
                    bias=b_sb[:, hp : hp + 1],
                )

            def emit_v_group(vh, si):
                # V projection for head pair vh (2 heads, 128 channels)
                ch = slice(vh * 128, (vh + 1) * 128)
                psv = psum.tile([128, 128], F32, tag="s", bufs=2,
                                name=f"psv{vh}_{si}")
                for f in range(NF):
                    nc.tensor.matmul(
                        psv[:],
                        lhsT=xtv[f][:, si * 128 : (si + 1) * 128],
                        rhs=wv_sb[:, f, ch],
                        start=(f == 0),
                        stop=False,
                    )
                nc.tensor.matmul(
                    psv[:], lhsT=ones1[:], rhs=bv_sb[:, ch], start=False,
                    stop=True,
                )
                nc.vector.tensor_copy(V[:, 2 * vh, si, 0:HD], psv[:, 0:HD])
                nc.scalar.copy(V[:, 2 * vh + 1, si, 0:HD], psv[:, HD:128])

            # --- upfront: K pair0, Q pair0, V heads 0/1 ---
            emit_qk_outer(xtk, wk_sb, bk_sb, KT, 0, "k0")
            emit_qk_outer(xtq, wq_sb, bq_sb, QT, 0, "q0")
            for si in range(NSI):
                emit_v_group(0, si)

            # --- attention, software-pipelined; pair-1 projections and V
            # heads 2/3 interleaved into the stream ---
            units = [(qs, 0) for qs in range(NQS)] + [(qs, 1) for qs in range(NQS)]
            nsteps = len(units) * NSI
            po = {}
            e_t = {}
            ON = {}

            items = []
            for j in range(NSI):
                items.append((20 + 2 * j, lambda j=j: emit_v_group(1, j)))
            for sw in range(NQS):
                items.append((24 + 4 * sw,
                              lambda sw=sw: emit_qk_group(
                                  xtk, wk_sb, bk_sb, KT, 1, sw, "k1")))
            for sw in range(NQS):
                items.append((40 + 4 * sw,
                              lambda sw=sw: emit_qk_group(
                                  xtq, wq_sb, bq_sb, QT, 1, sw, "q1")))
            items.sort(key=lambda x: x[0])

            def emit_scores(t):
                u, si = divmod(t, NSI)
                qs, hp = units[u]
                qsl = slice(qs * 512, (qs + 1) * 512)
                ks = slice(si * 128, (si + 1) * 128)
                if si == 0:
                    po[u] = (
                        psum.tile([HD + 1, 512], F32, tag="po0", bufs=2,
                                  name=f"po0_{qs}{hp}"),
                        psum.tile([HD + 1, 512], F32, tag="po1", bufs=2,
                                  name=f"po1_{qs}{hp}"),
                    )
                ps = psum.tile([128, 1024], F32, tag="s", bufs=2,
                               name=f"ps_{qs}{hp}{si}")
                nc.tensor.matmul(
                    ps[:, 0:512],
                    lhsT=KT[hp][0:64, ks],
                    rhs=QT[hp][0:64, qsl],
                    tile_position=(0, 0),
                )
                nc.tensor.matmul(
                    ps[:, 512:1024],
                    lhsT=KT[hp][64:128, ks],
                    rhs=QT[hp][64:128, qsl],
                    tile_position=(64, 0),
                )
                e = epool.tile([128, 1024], BF, tag="e", name=f"e_{qs}{hp}{si}")
                nc.scalar.activation(e[:], ps[:], Exp, scale=0.125)
                e_t[t] = e

            def emit_attnv(t):
                u, si = divmod(t, NSI)
                qs, hp = units[u]
                h0, h1 = 2 * hp, 2 * hp + 1
                po0, po1 = po[u]
                e = e_t.pop(t)
                nc.tensor.matmul(
                    po0[:], lhsT=V[:, h0, si, :], rhs=e[:, 0:512],
                    start=(si == 0), stop=(si == NSI - 1),
                )
                nc.tensor.matmul(
                    po1[:], lhsT=V[:, h1, si, :], rhs=e[:, 512:1024],
                    start=(si == 0), stop=(si == NSI - 1),
                )
                if si == NSI - 1:
                    emit_norm(u, split=(u == len(units) - 1))
                    if hp == 1:
                        pending_wo.append([qs, t + LAG_WO])

            def emit_norm(u, split=False):
                qs, hp = units[u]
                po0, po1 = po.pop(u)
                rc0 = npool.tile([HD + 1, 512], F32, tag="rc0", name=f"rc0_{qs}{hp}")
                rc1 = npool.tile([HD + 1, 512], F32, tag="rc1", name=f"rc1_{qs}{hp}")
                ri0 = npool.tile([1, 512], F32, tag="ri0", name=f"ri0_{qs}{hp}")
                ri1 = npool.tile([1, 512], F32, tag="ri1", name=f"ri1_{qs}{hp}")
                rb0 = npool.tile([HD, 512], F32, tag="rb0", name=f"rb0_{qs}{hp}")
                rb1 = npool.tile([HD, 512], F32, tag="rb1", name=f"rb1_{qs}{hp}")
                on = onpool.tile([128, 512], BF, tag=f"on{hp}",
                                 bufs=(4 if hp == 0 else 2), name=f"on_{qs}{hp}")
                on1 = onpool.tile([HD, 512], BF, tag="onx", name=f"onx_{qs}{hp}")
                halves = (
                    (slice(0, 256), slice(256, 512)) if split else (slice(0, 512),)
                )
                for cl in halves:
                    nc.vector.reciprocal(rc0[HD : HD + 1, cl], po0[HD : HD + 1, cl])
                    nc.gpsimd.dma_start(ri0[0:1, cl], rc0[HD : HD + 1, cl])
                    nc.gpsimd.partition_broadcast(rb0[:, cl], ri0[0:1, cl])
                    nc.vector.tensor_mul(on[0:HD, cl], po0[0:HD, cl], rb0[:, cl])
                for cl in halves:
                    nc.vector.reciprocal(rc1[HD : HD + 1, cl], po1[HD : HD + 1, cl])
                    nc.gpsimd.dma_start(ri1[0:1, cl], rc1[HD : HD + 1, cl])
                    nc.gpsimd.partition_broadcast(rb1[:, cl], ri1[0:1, cl])
                    nc.vector.tensor_mul(on1[:, cl], po1[0:HD, cl], rb1[:, cl])
                nc.gpsimd.dma_start(on[HD:128, :], on1[:])
                ON.setdefault(qs, {})[hp] = on

            def emit_wo(qs):
                qsl = slice(qs * 512, (qs + 1) * 512)
                on_by_hp = ON.pop(qs)
                for oc in range(NF):
                    py = psum.tile([128, 512], F32, tag="s", bufs=2,
                                   name=f"py_{qs}{oc}")
                    nc.tensor.matmul(
                        py[:], lhsT=wo_sb[:, 0, oc, :], rhs=on_by_hp[0][:],
                        start=True, stop=False,
                    )
                    nc.tensor.matmul(
                        py[:], lhsT=wo_sb[:, 1, oc, :], rhs=on_by_hp[1][:],
                        start=False, stop=True,
                    )
                    ysb = ypool.tile([128, 512], BF, tag="y", name=f"y_{qs}{oc}")
                    if qs == NQS - 1:
                        nc.scalar.copy(ysb[:], py[:])
                    else:
                        nc.vector.tensor_copy(ysb[:], py[:])
                    nc.sync.dma_start(y_ap[oc, :, qsl], ysb[:])

            pending_wo = []
            item_i = 0
            for t in range(nsteps + LAG):
                if t < nsteps:
                    emit_scores(t)
                if t >= LAG:
                    emit_attnv(t - LAG)
                while pending_wo and pending_wo[0][1] <= t - LAG:
                    emit_wo(pending_wo.pop(0)[0])
                while item_i < len(items) and items[item_i][0] <= t:
                    items[item_i][1]()
                    item_i += 1
            while item_i < len(items):
                items[item_i][1]()
                item_i += 1
            while pending_wo:
                emit_wo(pending_wo.pop(0)[0])
    nc.compile()
    return nc


def _get_nc():
    global _nc
    with _cache:
        if _nc is None:
            _nc = _build_nc()
        return _nc


def kernel(q, k, v, wq_w, wq_b, wk_w, wk_b, wv_w, wv_b, wo_w, wo_b):
    global LAST_RESULT
    nc = _get_nc()

    def xT(a, b):
        return np.ascontiguousarray(np.asarray(a)[b].astype(BF16).T)

    xs = {
        "xq_t": [xT(q, b) for b in range(B)],
        "xk_t": [xT(k, b) for b in range(B)],
        "xv_t": [xT(v, b) for b in range(B)],
    }
    wq_w = np.asarray(wq_w, dtype=np.float32)
    wk_w = np.asarray(wk_w, dtype=np.float32)
    wv_w = np.asarray(wv_w, dtype=np.float32)
    wo_w = np.asarray(wo_w, dtype=np.float32)

    in_maps = []
    for c in range(N_CORES):
        b, hg = c // 4, c % 4
        cs = slice(hg * CPC, (hg + 1) * CPC)
        in_maps.append({
            "xq_t": xs["xq_t"][b],
            "xk_t": xs["xk_t"][b],
            "xv_t": xs["xv_t"][b],
            "wq_t": np.ascontiguousarray(wq_w[cs, :].astype(BF16).T),
            "wk_t": np.ascontiguousarray(wk_w[cs, :].astype(BF16).T),
            "wv_t": np.ascontiguousarray(wv_w[cs, :].astype(BF16).T),
            "bq": np.asarray(wq_b, np.float32)[cs].reshape(CPC, 1),
            "bk": np.asarray(wk_b, np.float32)[cs].reshape(CPC, 1),
            "bv": np.asarray(wv_b, np.float32)[cs].astype(BF16).reshape(1, CPC),
            "wo_t": np.ascontiguousarray(wo_w[:, cs].astype(BF16).T),
        })

    res = run_bass_kernel_spmd(
        nc, in_maps, core_ids=list(range(N_CORES)),
        trace=bool(int(os.environ.get("MHA_TRACE", "0"))),
    )
    LAST_RESULT = res

    out = np.empty((B, S, H), dtype=np.float32)
    bias = np.asarray(wo_b, np.float64)[None, :]
    for b in range(B):
        y = res.results[4 * b]["y_t"].astype(np.float64)
        for hg in range(1, 4):
            y += res.results[4 * b + hg]["y_t"]
        out[b] = (y.T + bias).astype(np.float32)
    return out


# revision 21
# speedup vs baseline: 1.1181x; 1.1181x over previous
"""Multi-head attention (B=2, S=2048, H=1024, 16 heads) on 8 trn2 NeuronCores.

Sharding: 2-way batch x 4-way head tensor parallel. Core c owns batch c//4 and
heads 4*(c%4) .. 4*(c%4)+4 (256 channels of the QKV projections, 256 input
channels of the output projection). Each core consumes its batch's activations
(transposed, bf16) and returns a bf16 partial of the wo projection; the host
sums the 4 partials per batch and adds the bias.

The attention inner loop is bound by the scalar-engine exp (~1.05us per
128-key step), not the PE, so the kernel keeps only the minimum projection
work ahead of the attention stream (K/Q for head-pair 0, V for heads 0-1,
DMA-paced f-outer) and interleaves the remaining projections (Q/K pair 1,
V heads 2-3) into the attention steps to fill the PE idle and keep the HAM
clock warm. attnV lags scores/exp by LAG steps; wo is deferred LAG_WO steps
so the softmax-norm chain (reciprocal+broadcast) runs in the shadow of the
next unit.
"""

import os
import threading

import numpy as np
import ml_dtypes

import concourse.bass as bass
import concourse.mybir as mybir
import concourse.tile as tile
from concourse import bacc
from concourse.bass_utils import run_bass_kernel_spmd

BF16 = ml_dtypes.bfloat16
F32 = mybir.dt.float32
BF = mybir.dt.bfloat16

B = 2
S = 2048            # tokens per core (one batch)
H = 1024
NH_LOCAL = 4        # heads per core
HD = 64
CPC = NH_LOCAL * HD  # 256 channels per core
NF = H // 128       # feature chunks of the input dim
N_CORES = 8
LAG = 3             # attnV lag (in key-block steps) behind scores/exp
LAG_WO = 12         # wo deferral (steps) to hide the softmax-norm latency

_cache = threading.Lock()
_nc = None

LAST_RESULT = None  # BassKernelResults of the most recent run (for test.py)


def _build_nc():
    nc = bacc.Bacc(None, target_bir_lowering=False, debug=False)

    xq_d = nc.dram_tensor("xq_t", [H, S], BF, kind="ExternalInput")
    xk_d = nc.dram_tensor("xk_t", [H, S], BF, kind="ExternalInput")
    xv_d = nc.dram_tensor("xv_t", [H, S], BF, kind="ExternalInput")
    wq_d = nc.dram_tensor("wq_t", [H, CPC], BF, kind="ExternalInput")
    wk_d = nc.dram_tensor("wk_t", [H, CPC], BF, kind="ExternalInput")
    wv_d = nc.dram_tensor("wv_t", [H, CPC], BF, kind="ExternalInput")
    bq_d = nc.dram_tensor("bq", [CPC, 1], F32, kind="ExternalInput")
    bk_d = nc.dram_tensor("bk", [CPC, 1], F32, kind="ExternalInput")
    bv_d = nc.dram_tensor("bv", [1, CPC], BF, kind="ExternalInput")
    wo_d = nc.dram_tensor("wo_t", [CPC, H], BF, kind="ExternalInput")
    y_d = nc.dram_tensor("y_t", [H, S], BF, kind="ExternalOutput")

    xq_ap = xq_d.rearrange("(nf p) s -> nf p s", p=128)
    xk_ap = xk_d.rearrange("(nf p) s -> nf p s", p=128)
    xv_ap = xv_d.rearrange("(nf p) s -> nf p s", p=128)
    wq_ap = wq_d.rearrange("(nf p) c -> nf p c", p=128)
    wk_ap = wk_d.rearrange("(nf p) c -> nf p c", p=128)
    wv_ap = wv_d.rearrange("(nf p) c -> nf p c", p=128)
    y_ap = y_d.rearrange("(no p) s -> no p s", p=128)

    Exp = mybir.ActivationFunctionType.Exp
    Copy = mybir.ActivationFunctionType.Identity

    NSI = S // 128   # 16 key blocks
    NQS = S // 512   # 4 query slices

    with tile.TileContext(nc) as tc:
        with (
            tc.tile_pool(name="const", bufs=1) as const,
            tc.tile_pool(name="res", bufs=1) as res,
            tc.tile_pool(name="psum", bufs=1, space="PSUM") as psum,
            tc.tile_pool(name="epool", bufs=6) as epool,
            tc.tile_pool(name="npool", bufs=2) as npool,
            tc.tile_pool(name="onpool", bufs=2) as onpool,
            tc.tile_pool(name="ypool", bufs=3) as ypool,
            tc.tile_pool(name="xin", bufs=8) as xin,
        ):
            # --- input chunk DMAs first (K, Q, V order), on the SP queue ---
            xtk, xtq, xtv = [], [], []
            for f in range(NF):
                t = xin.tile([128, S], BF, tag="xk", name=f"xk{f}")
                nc.sync.dma_start(t[:], xk_ap[f])
                xtk.append(t)
            for f in range(NF):
                t = xin.tile([128, S], BF, tag="xq", name=f"xq{f}")
                nc.sync.dma_start(t[:], xq_ap[f])
                xtq.append(t)
            for f in range(NF):
                t = xin.tile([128, S], BF, tag="xv", name=f"xv{f}")
                nc.sync.dma_start(t[:], xv_ap[f])
                xtv.append(t)

            # --- weights (per-chunk, on the ACT/gpsimd issue queues) ---
            wq_sb = const.tile([128, NF, CPC], BF)
            wk_sb = const.tile([128, NF, CPC], BF)
            wv_sb = const.tile([128, NF, CPC], BF)
            wo_sb = const.tile([128, 2, NF, 128], BF)
            bq_sb = const.tile([128, 2], F32)
            bk_sb = const.tile([128, 2], F32)
            bv_sb = const.tile([1, CPC], BF)
            ones1 = const.tile([1, 128], BF)
            for f in range(NF):
                nc.scalar.dma_start(wk_sb[:, f, :], wk_ap[f])
            nc.scalar.dma_start(
                bk_sb[:], bk_d.rearrange("(hp p) one -> p (hp one)", p=128)
            )
            for f in range(NF):
                nc.scalar.dma_start(wq_sb[:, f, :], wq_ap[f])
            nc.scalar.dma_start(
                bq_sb[:], bq_d.rearrange("(hp p) one -> p (hp one)", p=128)
            )
            for f in range(NF):
                nc.gpsimd.dma_start(wv_sb[:, f, :], wv_ap[f])
            nc.gpsimd.dma_start(bv_sb[:], bv_d[:])
            nc.scalar.dma_start(
                wo_sb[:], wo_d.rearrange("(hp p) (no c) -> p hp no c", p=128, c=128)
            )
            nc.gpsimd.memset(ones1[:], 1.0)

            # --- residents ---
            QT = [res.tile([128, S], BF, name=f"QT{hp}") for hp in range(2)]
            KT = [res.tile([128, S], BF, name=f"KT{hp}") for hp in range(2)]
            V = res.tile([128, NH_LOCAL, NSI, HD + 1], BF)
            nc.gpsimd.memset(V[:, :, :, HD : HD + 1], 1.0)

            # --- projection emitters ---
            def emit_qk_outer(xt, w_sb, b_sb, out_t, hp, pname):
                # f-outer over 4 live psum groups: starts as soon as the
                # first input chunk lands (DMA-paced)
                cs = slice(hp * 128, (hp + 1) * 128)
                pst = []
                for sw in range(NQS):
                    tag = ("s", "s", "pw", "pw")[sw]
                    pst.append(psum.tile([128, 512], F32, tag=tag, bufs=2,
                                         name=f"pj{pname}{sw}"))
                for f in range(NF):
                    for sw in range(NQS):
                        nc.tensor.matmul(
                            pst[sw][:],
                            lhsT=w_sb[:, f, cs],
                            rhs=xt[f][:, sw * 512 : (sw + 1) * 512],
                            start=(f == 0),
                            stop=(f == NF - 1),
                        )
                for sw in range(NQS):
                    nc.scalar.activation(
                        out_t[hp][:, sw * 512 : (sw + 1) * 512], pst[sw][:],
                        Copy, bias=b_sb[:, hp : hp + 1],
                    )

            def emit_qk_group(xt, w_sb, b_sb, out_t, hp, sw, pname):
                # single f-inner group (used for the interleaved pair-1 work)
                cs = slice(hp * 128, (hp + 1) * 128)
                ps = psum.tile([128, 512], F32, tag="pw", bufs=2,
                               name=f"pg{pname}{hp}{sw}")
                for f in range(NF):
                    nc.tensor.matmul(
                        ps[:],
                        lhsT=w_sb[:, f, cs],
                        rhs=xt[f][:, sw * 512 : (sw + 1) * 512],
                        start=(f == 0),
                        stop=(f == NF - 1),
                    )
                nc.scalar.activation(
                    out_t[hp][:, sw * 512 : (sw + 1) * 512], ps[:], Copy,
                    bias=b_sb[:, hp : hp + 1],
                )

            def emit_v_group(vh, si):
                # V projection for head pair vh (2 heads, 128 channels)
                ch = slice(vh * 128, (vh + 1) * 128)
                psv = psum.tile([128, 128], F32, tag="pw", bufs=2,
                                name=f"psv{vh}_{si}")
                for f in range(NF):
                    nc.tensor.matmul(
                        psv[:],
                        lhsT=xtv[f][:, si * 128 : (si + 1) * 128],
                        rhs=wv_sb[:, f, ch],
                        start=(f == 0),
                        stop=False,
                    )
                nc.tensor.matmul(
                    psv[:], lhsT=ones1[:], rhs=bv_sb[:, ch], start=False,
                    stop=True,
                )
                nc.vector.tensor_copy(V[:, 2 * vh, si, 0:HD], psv[:, 0:HD])
                nc.scalar.copy(V[:, 2 * vh + 1, si, 0:HD], psv[:, HD:128])

            # --- upfront: K pair0, Q pair0, V heads 0/1 ---
            emit_qk_outer(xtk, wk_sb, bk_sb, KT, 0, "k0")
            emit_qk_outer(xtq, wq_sb, bq_sb, QT, 0, "q0")
            for si in range(NSI):
                emit_v_group(0, si)

            # --- attention, software-pipelined; pair-1 projections and V
            # heads 2/3 interleaved into the stream ---
            units = [(qs, 0) for qs in range(NQS)] + [(qs, 1) for qs in range(NQS)]
            nsteps = len(units) * NSI
            po = {}
            e_t = {}
            ON = {}

            items = []
            for j in range(NSI):
                items.append((20 + 2 * j, lambda j=j: emit_v_group(1, j)))
            for sw in range(NQS):
                items.append((24 + 4 * sw,
                              lambda sw=sw: emit_qk_group(
                                  xtk, wk_sb, bk_sb, KT, 1, sw, "k1")))
            for sw in range(NQS):
                items.append((40 + 4 * sw,
                              lambda sw=sw: emit_qk_group(
                                  xtq, wq_sb, bq_sb, QT, 1, sw, "q1")))
            items.sort(key=lambda x: x[0])

            def emit_scores(t):
                u, si = divmod(t, NSI)
                qs, hp = units[u]
                qsl = slice(qs * 512, (qs + 1) * 512)
                ks = slice(si * 128, (si + 1) * 128)
                if si == 0:
                    po[u] = (
                        psum.tile([HD + 1, 512], F32, tag="po0", bufs=1,
                                  name=f"po0_{qs}{hp}"),
                        psum.tile([HD + 1, 512], F32, tag="po1", bufs=1,
                                  name=f"po1_{qs}{hp}"),
                    )
                ps = psum.tile([128, 1024], F32, tag="s", bufs=2,
                               name=f"ps_{qs}{hp}{si}")
                nc.tensor.matmul(
                    ps[:, 0:512],
                    lhsT=KT[hp][0:64, ks],
                    rhs=QT[hp][0:64, qsl],
                    tile_position=(0, 0),
                )
                nc.tensor.matmul(
                    ps[:, 512:1024],
                    lhsT=KT[hp][64:128, ks],
                    rhs=QT[hp][64:128, qsl],
                    tile_position=(64, 0),
                )
                e = epool.tile([128, 1024], BF, tag="e", name=f"e_{qs}{hp}{si}")
                nc.scalar.activation(e[:], ps[:], Exp, scale=0.125)
                e_t[t] = e

            def emit_attnv(t):
                u, si = divmod(t, NSI)
                qs, hp = units[u]
                h0, h1 = 2 * hp, 2 * hp + 1
                po0, po1 = po[u]
                e = e_t.pop(t)
                nc.tensor.matmul(
                    po0[:], lhsT=V[:, h0, si, :], rhs=e[:, 0:512],
                    start=(si == 0), stop=(si == NSI - 1),
                )
                nc.tensor.matmul(
                    po1[:], lhsT=V[:, h1, si, :], rhs=e[:, 512:1024],
                    start=(si == 0), stop=(si == NSI - 1),
                )
                if si == NSI - 1:
                    emit_norm(u, split=(u == len(units) - 1))
                    if hp == 1:
                        pending_wo.append([qs, t + LAG_WO])

            def emit_norm(u, split=False):
                qs, hp = units[u]
                po0, po1 = po.pop(u)
                so0 = npool.tile([HD + 1, 512], F32, tag="so0", name=f"so0_{qs}{hp}")
                so1 = npool.tile([HD + 1, 512], F32, tag="so1", name=f"so1_{qs}{hp}")
                nc.vector.tensor_copy(so0[:], po0[:])
                nc.vector.tensor_copy(so1[:], po1[:])
                rc0 = npool.tile([HD + 1, 512], F32, tag="rc0", name=f"rc0_{qs}{hp}")
                rc1 = npool.tile([HD + 1, 512], F32, tag="rc1", name=f"rc1_{qs}{hp}")
                ri0 = npool.tile([1, 512], F32, tag="ri0", name=f"ri0_{qs}{hp}")
                ri1 = npool.tile([1, 512], F32, tag="ri1", name=f"ri1_{qs}{hp}")
                rb0 = npool.tile([HD, 512], F32, tag="rb0", name=f"rb0_{qs}{hp}")
                rb1 = npool.tile([HD, 512], F32, tag="rb1", name=f"rb1_{qs}{hp}")
                on = onpool.tile([128, 512], BF, tag=f"on{hp}",
                                 bufs=(4 if hp == 0 else 2), name=f"on_{qs}{hp}")
                on1 = onpool.tile([HD, 512], BF, tag="onx", name=f"onx_{qs}{hp}")
                halves = (
                    (slice(0, 256), slice(256, 512)) if split else (slice(0, 512),)
                )
                for cl in halves:
                    nc.vector.reciprocal(rc0[HD : HD + 1, cl], so0[HD : HD + 1, cl])
                    nc.gpsimd.dma_start(ri0[0:1, cl], rc0[HD : HD + 1, cl])
                    nc.gpsimd.partition_broadcast(rb0[:, cl], ri0[0:1, cl])
                    nc.vector.tensor_mul(on[0:HD, cl], so0[0:HD, cl], rb0[:, cl])
                for cl in halves:
                    nc.vector.reciprocal(rc1[HD : HD + 1, cl], so1[HD : HD + 1, cl])
                    nc.gpsimd.dma_start(ri1[0:1, cl], rc1[HD : HD + 1, cl])
                    nc.gpsimd.partition_broadcast(rb1[:, cl], ri1[0:1, cl])
                    nc.vector.tensor_mul(on1[:, cl], so1[0:HD, cl], rb1[:, cl])
                nc.gpsimd.dma_start(on[HD:128, :], on1[:])
                ON.setdefault(qs, {})[hp] = on

            def emit_wo(qs):
                qsl = slice(qs * 512, (qs + 1) * 512)
                on_by_hp = ON.pop(qs)
                for oc in range(NF):
                    py = psum.tile([128, 512], F32, tag="pw", bufs=2,
                                   name=f"py_{qs}{oc}")
                    nc.tensor.matmul(
                        py[:], lhsT=wo_sb[:, 0, oc, :], rhs=on_by_hp[0][:],
                        start=True, stop=False,
                    )
                    nc.tensor.matmul(
                        py[:], lhsT=wo_sb[:, 1, oc, :], rhs=on_by_hp[1][:],
                        start=False, stop=True,
                    )
                    ysb = ypool.tile([128, 512], BF, tag="y", name=f"y_{qs}{oc}")
                    if qs == NQS - 1:
                        nc.scalar.copy(ysb[:], py[:])
                    else:
                        nc.vector.tensor_copy(ysb[:], py[:])
                    nc.sync.dma_start(y_ap[oc, :, qsl], ysb[:])

            pending_wo = []
            item_i = 0
            for t in range(nsteps + LAG):
                if t < nsteps:
                    emit_scores(t)
                if t >= LAG:
                    emit_attnv(t - LAG)
                while pending_wo and pending_wo[0][1] <= t - LAG:
                    emit_wo(pending_wo.pop(0)[0])
                while item_i < len(items) and items[item_i][0] <= t:
                    items[item_i][1]()
                    item_i += 1
            while item_i < len(items):
                items[item_i][1]()
                item_i += 1
            while pending_wo:
                emit_wo(pending_wo.pop(0)[0])
    nc.compile()
    return nc


def _get_nc():
    global _nc
    with _cache:
        if _nc is None:
            _nc = _build_nc()
        return _nc


def kernel(q, k, v, wq_w, wq_b, wk_w, wk_b, wv_w, wv_b, wo_w, wo_b):
    global LAST_RESULT
    nc = _get_nc()

    def xT(a, b):
        return np.ascontiguousarray(np.asarray(a)[b].astype(BF16).T)

    xs = {
        "xq_t": [xT(q, b) for b in range(B)],
        "xk_t": [xT(k, b) for b in range(B)],
        "xv_t": [xT(v, b) for b in range(B)],
    }
    wq_w = np.asarray(wq_w, dtype=np.float32)
    wk_w = np.asarray(wk_w, dtype=np.float32)
    wv_w = np.asarray(wv_w, dtype=np.float32)
    wo_w = np.asarray(wo_w, dtype=np.float32)

    in_maps = []
    for c in range(N_CORES):
        b, hg = c // 4, c % 4
        cs = slice(hg * CPC, (hg + 1) * CPC)
        in_maps.append({
            "xq_t": xs["xq_t"][b],
            "xk_t": xs["xk_t"][b],
            "xv_t": xs["xv_t"][b],
            "wq_t": np.ascontiguousarray(wq_w[cs, :].astype(BF16).T),
            "wk_t": np.ascontiguousarray(wk_w[cs, :].astype(BF16).T),
            "wv_t": np.ascontiguousarray(wv_w[cs, :].astype(BF16).T),
            "bq": np.asarray(wq_b, np.float32)[cs].reshape(CPC, 1),
            "bk": np.asarray(wk_b, np.float32)[cs].reshape(CPC, 1),
            "bv": np.asarray(wv_b, np.float32)[cs].astype(BF16).reshape(1, CPC),
            "wo_t": np.ascontiguousarray(wo_w[:, cs].astype(BF16).T),
        })

    res = run_bass_kernel_spmd(
        nc, in_maps, core_ids=list(range(N_CORES)),
        trace=bool(int(os.environ.get("MHA_TRACE", "0"))),
    )
    LAST_RESULT = res

    out = np.empty((B, S, H), dtype=np.float32)
    bias = np.asarray(wo_b, np.float64)[None, :]
    for b in range(B):
        y = res.results[4 * b]["y_t"].astype(np.float64)
        for hg in range(1, 4):
            y += res.results[4 * b + hg]["y_t"]
        out[b] = (y.T + bias).astype(np.float32)
    return out
